# revision 14
# baseline (speedup 1.0000x reference)
"""Trainium2 Bass kernel for the LSTM decoder problem.

Model (per reference):
    emb = emb_table[text]                       # [B, T, NE]
    x = concat(batch_H, emb)                    # [B, T, IN+NE]
    gx = einsum('bti,gi->tbg', x, W_ih) + b_ih  # [T, B, 4H]
    (h, c) LSTM recurrence over T=26 steps (PyTorch gate order i,f,g,o)
    probs = hs @ W_gen.T + b_gen
    returns (probs [B,T,NC], hs [B,T,H])

Strategy:
  - Data-parallel: batch 4096 -> 512 rows per core across 8 NeuronCores.
  - "Transposed" activation layout on-chip: gates/h/probs as [feature, batch]
    so h feeds the next step's matmul directly -- zero transposes anywhere on
    the device; batch_H shards are shipped [T, IN, BL] and the hid/probs
    outputs come back [T, feat, BL] and are transposed during host unshard.
  - Per-step fused gx: each gates psum tile [128g, 512b] accumulates
    4 batch_H K-chunks + 1 one-hot embedding matmul + 4 h K-chunks (t>0).
  - Embedding via one-hot matmul: G' = emb_table @ W_ih[:,IN:].T + b_ih + b_hh
    (host precompute, weight-space only); the one-hot is built on device from
    text via a K=1 broadcast matmul + is_equal against iota -- exact since
    one-hot entries are 0/1 and text values <= 97 are exact in bf16.
  - All matmuls bf16 (full 1 cyc/row PE rate, FWL weight loads). Keeping ANY
    fp32-class matmul in the program corrupts bf16 matmul weights
    (order-dependent -- the FWL-after-FP32-matmul hazard walrus only guards
    in compile order), so even the tiny broadcast matmul is bf16.
  - Gate nonlinearities on ScalarE from PSUM (sigmoid/tanh share one ACT
    table set); cell-state math in fp32 on VectorE; h stored fp32 for the
    output path with a separate bf16 copy feeding the recurrence matmuls.
  - Measured: ~863 us on hardware, PE ~95% busy at ~221 ns per N=512 matmul
    (216 ns streaming floor); outputs rel-err ~4e-3, resid_var ~1e-5.
"""
import sys

for _p in ("/opt/trn_rl_repo",):
    if _p not in sys.path:
        sys.path.insert(0, _p)

import numpy as np
import ml_dtypes

# --- NTFF profile hook shim: the image's antenv lacks axon_hooks; inject a
# module so run_bass_kernel_spmd(trace=True) can profile via the axon .so. ---
import types as _types

def _make_axon_hooks():
    mod = _types.ModuleType("antenv.axon_hooks")
    state = {"hook": None}

    def set_axon_ntff_profile_hook(hook):
        state["hook"] = hook

    def get_axon_ntff_profile_hook():
        if state["hook"] is None:
            try:
                if "/root/.axon_site" not in sys.path:
                    sys.path.insert(0, "/root/.axon_site")
                from trn_agent_boot.trn_boot import _ntff_profile_via_ctypes
                state["hook"] = _ntff_profile_via_ctypes("/opt/axon/libaxon_pjrt.so")
            except Exception:
                return None
        return state["hook"]

    mod.set_axon_ntff_profile_hook = set_axon_ntff_profile_hook
    mod.get_axon_ntff_profile_hook = get_axon_ntff_profile_hook
    return mod

if "antenv.axon_hooks" not in sys.modules:
    sys.modules["antenv.axon_hooks"] = _make_axon_hooks()

import concourse.bacc as bacc
import concourse.mybir as mybir
import concourse.tile as tile
from concourse.bass_utils import run_bass_kernel_spmd

B, T, IN, H, NC_, NE = 4096, 26, 512, 512, 97, 256
NCORES = 8
BL = B // NCORES      # 512 batch rows per core
G4 = 4 * H            # 2048 gate width
NCV = NC_ + 1         # 98 vocab (embedding rows)
KI = IN // 128        # 4 K-chunks over encoder features
KH = H // 128         # 4 K-chunks over hidden
MB = BL // 128        # 4 batch partition tiles
GT = G4 // 128        # 16 gate partition tiles

F32 = mybir.dt.float32
F32R = mybir.dt.float32r
I32 = mybir.dt.int32
AF = mybir.ActivationFunctionType
OP = mybir.AluOpType

USE_BF16 = True
BF16 = mybir.dt.bfloat16
MMD = BF16 if USE_BF16 else F32R

TRACE = False
LAST_RESULT = None
_NC = None


def _build():
    nc = bacc.Bacc("TRN2", target_bir_lowering=False, debug=False,
                   enable_asserts=True, num_devices=NCORES)

    bh_d = nc.dram_tensor("bh", [T, IN, BL], MMD, kind="ExternalInput")
    textT_d = nc.dram_tensor("textT", [1, T, BL], MMD, kind="ExternalInput")
    wx_d = nc.dram_tensor("wx", [KI, 128, G4], MMD, kind="ExternalInput")
    whh_d = nc.dram_tensor("whh", [KH, 128, G4], MMD, kind="ExternalInput")
    gp_d = nc.dram_tensor("gp", [NCV, G4], MMD, kind="ExternalInput")
    wg_d = nc.dram_tensor("wg", [KH, 128, NC_], MMD, kind="ExternalInput")
    bgen_d = nc.dram_tensor("bgen", [NC_, 1], F32, kind="ExternalInput")
    iota_d = nc.dram_tensor("iota", [NCV, 1], F32, kind="ExternalInput")
    ones_d = nc.dram_tensor("ones", [1, NCV], MMD, kind="ExternalInput")
    
    probs_d = nc.dram_tensor("probs", [T, NC_, BL], F32, kind="ExternalOutput")
    hid_d = nc.dram_tensor("hid", [T, H, BL], F32, kind="ExternalOutput")

    with tile.TileContext(nc) as tc:
        with (
            tc.tile_pool(name="wpool", bufs=1) as wp,
            tc.tile_pool(name="sb", bufs=2) as sb,
            tc.tile_pool(name="ps", bufs=1, space="PSUM") as ps,
        ):
            # ---- prologue: step-0 inputs first so PE can start ASAP;
            # whh is not needed until t=1, so its DMA goes last. ----
            iota = wp.tile([NCV, 1], F32)
            nc.sync.dma_start(iota[:], iota_d.ap())
            ones = wp.tile([1, NCV], MMD)
            nc.sync.dma_start(ones[:], ones_d.ap())
            bhT0 = sb.tile([128, KI, BL], MMD, tag="bhT", bufs=2)
            nc.sync.dma_start(
                bhT0[:], bh_d.ap()[0].rearrange("(k p) b -> p k b", k=KI))
            txt0 = sb.tile([1, BL], MMD, tag="txt", bufs=2)
            nc.sync.dma_start(txt0[:], textT_d.ap()[:, 0, :])
            wx = wp.tile([128, KI, G4], MMD)
            for k in range(KI):
                nc.sync.dma_start(wx[:, k, :], wx_d.ap()[k])
            gp = wp.tile([NCV, G4], MMD)
            nc.sync.dma_start(gp[:], gp_d.ap())
            wg = wp.tile([128, KH, NC_], MMD)
            nc.sync.dma_start(wg[:], wg_d.ap().rearrange("k p c -> p k c"))
            bgen = wp.tile([NC_, 1], F32)
            nc.sync.dma_start(bgen[:], bgen_d.ap())
            whh = wp.tile([128, KH, G4], MMD)
            for k in range(KH):
                nc.sync.dma_start(whh[:, k, :], whh_d.ap()[k])


            h_prev = None
            c_prev = None
            hm_prev = None
            for t in range(T):
                # ---- load batch_H_T[t] directly in matmul layout [i, b] ----
                if t == 0:
                    bhT = bhT0
                    txt = txt0
                else:
                    bhT = sb.tile([128, KI, BL], MMD, tag="bhT", bufs=2)
                    nc.sync.dma_start(
                        bhT[:], bh_d.ap()[t].rearrange("(k p) b -> p k b", k=KI))
                    txt = sb.tile([1, BL], MMD, tag="txt", bufs=2)
                    nc.sync.dma_start(txt[:], textT_d.ap()[:, t, :])

                # ---- one-hot of text[:, t] over vocab ----
                bc = ps.tile([NCV, BL], F32, tag="tbc", bufs=1)
                nc.tensor.matmul(bc[:], ones[:], txt[:])
                oh = sb.tile([NCV, BL], MMD, tag="oh", bufs=2)
                nc.vector.tensor_scalar(oh[:], bc[:], iota[:], None, OP.is_equal)

                # ---- gates: [2048g, 512b] in 16 psum tiles of [128, 512] ----
                # gate order along g: i(0-3) f(4-7) g(8-11) o(12-15)
                sg = {}
                for gt in range(GT):
                    gs = slice(128 * gt, 128 * (gt + 1))
                    acc = ps.tile([128, BL], F32, tag="gates", bufs=6)
                    for k in range(KI):
                        nc.tensor.matmul(acc[:], wx[:, k, gs], bhT[:, k, :],
                                         start=(k == 0), stop=False)
                    nc.tensor.matmul(acc[:], gp[:, gs], oh[:],
                                     start=False, stop=(t == 0))
                    if t > 0:
                        for k in range(KH):
                            nc.tensor.matmul(acc[:], whh[:, k, gs], hm_prev[k][:],
                                             start=False, stop=(k == KH - 1))
                    func = AF.Tanh if gt // 4 == 2 else AF.Sigmoid
                    s = sb.tile([128, BL], F32, tag="sg", bufs=16)
                    nc.scalar.activation(s[:], acc[:], func)
                    sg[gt] = s

                # ---- elementwise LSTM cell update per h-chunk ----
                h_new = []
                c_new = []
                hm_new = []
                for j in range(KH):
                    si, sf, tg, so = sg[j], sg[4 + j], sg[8 + j], sg[12 + j]
                    cn = sb.tile([128, BL], F32, tag="c", bufs=8)
                    if t == 0:
                        nc.vector.tensor_tensor(cn[:], si[:], tg[:], OP.mult)
                    else:
                        t1 = sb.tile([128, BL], F32, tag="t1", bufs=2)
                        nc.vector.tensor_tensor(t1[:], sf[:], c_prev[j][:], OP.mult)
                        t2 = sb.tile([128, BL], F32, tag="t2", bufs=2)
                        nc.vector.tensor_tensor(t2[:], si[:], tg[:], OP.mult)
                        nc.vector.tensor_tensor(cn[:], t1[:], t2[:], OP.add)
                    tc_ = sb.tile([128, BL], F32, tag="tc", bufs=3)
                    nc.scalar.activation(tc_[:], cn[:], AF.Tanh)
                    h = sb.tile([128, BL], F32, tag="h", bufs=8)
                    nc.vector.tensor_tensor(h[:], so[:], tc_[:], OP.mult)
                    hm = sb.tile([128, BL], MMD, tag="hm", bufs=8)
                    nc.vector.tensor_copy(hm[:], h[:])
                    h_new.append(h)
                    c_new.append(cn)
                    hm_new.append(hm)

                # ---- probs[t] = (W_gen @ h + b_gen) in [c, b] layout ----
                pp = ps.tile([NC_, BL], F32, tag="pp", bufs=1)
                for j in range(KH):
                    nc.tensor.matmul(pp[:], wg[:, j, :], hm_new[j][:],
                                     start=(j == 0), stop=(j == KH - 1))
                pb = sb.tile([NC_, BL], F32, tag="pb", bufs=2)
                nc.vector.tensor_scalar(pb[:], pp[:], bgen[:], None, OP.add)
                nc.sync.dma_start(probs_d.ap()[t], pb[:])

                # ---- hs[t] stored transposed [h, b]; host unshard transposes ----
                for j in range(KH):
                    nc.sync.dma_start(
                        hid_d.ap()[t, 128 * j:128 * (j + 1), :], h_new[j][:])

                h_prev, c_prev, hm_prev = h_new, c_new, hm_new

    nc.compile()
    return nc


def _get_nc():
    global _NC
    if _NC is None:
        _NC = _build()
    return _NC


def kernel(batch_H, text, emb_table, W_ih, W_hh, b_ih, b_hh, W_gen, b_gen):
    global LAST_RESULT
    batch_H = np.asarray(batch_H, dtype=np.float32)
    text = np.asarray(text, dtype=np.int32)
    emb_table = np.asarray(emb_table, dtype=np.float32)
    W_ih = np.asarray(W_ih, dtype=np.float32)
    W_hh = np.asarray(W_hh, dtype=np.float32)
    b_ih = np.asarray(b_ih, dtype=np.float32)
    b_hh = np.asarray(b_hh, dtype=np.float32)
    W_gen = np.asarray(W_gen, dtype=np.float32)
    b_gen = np.asarray(b_gen, dtype=np.float32)

    # weight-space host precompute (batch-independent)
    mmnp = ml_dtypes.bfloat16 if USE_BF16 else np.float32
    wx_h = np.ascontiguousarray(W_ih[:, :IN].T.reshape(KI, 128, G4)).astype(mmnp)
    whh_h = np.ascontiguousarray(W_hh.T.reshape(KH, 128, G4)).astype(mmnp)
    gp_h = np.ascontiguousarray(
        emb_table @ W_ih[:, IN:].T + b_ih[None, :] + b_hh[None, :]).astype(mmnp)
    wg_h = np.ascontiguousarray(W_gen.T.reshape(KH, 128, NC_)).astype(mmnp)
    bgen_h = np.ascontiguousarray(b_gen.reshape(NC_, 1))
    iota_h = np.arange(NCV, dtype=np.float32).reshape(NCV, 1)
    ones_h = np.ones((1, NCV), dtype=mmnp)

    in_maps = []
    for c in range(NCORES):
        bsl = slice(c * BL, (c + 1) * BL)
        in_maps.append(dict(
            bh=np.ascontiguousarray(batch_H[bsl].transpose(1, 2, 0)).astype(mmnp),
            textT=np.ascontiguousarray(text[bsl].T.astype(mmnp)).reshape(1, T, BL),
            wx=wx_h, whh=whh_h, gp=gp_h, wg=wg_h, bgen=bgen_h,
            iota=iota_h, ones=ones_h,
        ))

    nc = _get_nc()
    res = run_bass_kernel_spmd(nc, in_maps, list(range(NCORES)), trace=TRACE)
    LAST_RESULT = res

    # device layout is [T, C|H, BL]; unshard transposes back to [B, T, C|H]
    probs = np.concatenate(
        [res.results[c]["probs"].transpose(2, 0, 1) for c in range(NCORES)], axis=0)
    hid = np.concatenate(
        [res.results[c]["hid"].transpose(2, 0, 1) for c in range(NCORES)], axis=0)
    return np.ascontiguousarray(probs, dtype=np.float32), np.ascontiguousarray(hid, dtype=np.float32)


# revision 15
# speedup vs baseline: 1.0084x; 1.0084x over previous
"""Trainium2 Bass kernel for the LSTM decoder problem.

Model (per reference):
    emb = emb_table[text]                       # [B, T, NE]
    x = concat(batch_H, emb)                    # [B, T, IN+NE]
    gx = einsum('bti,gi->tbg', x, W_ih) + b_ih  # [T, B, 4H]
    (h, c) LSTM recurrence over T=26 steps (PyTorch gate order i,f,g,o)
    probs = hs @ W_gen.T + b_gen
    returns (probs [B,T,NC], hs [B,T,H])

Strategy:
  - Data-parallel: batch 4096 -> 512 rows per core across 8 NeuronCores.
  - "Transposed" activation layout on-chip: gates/h/probs as [feature, batch]
    so h feeds the next step's matmul directly -- zero transposes anywhere on
    the device; batch_H shards are shipped [T, IN, BL] and the hid/probs
    outputs come back [T, feat, BL] and are transposed during host unshard.
  - Per-step fused gx: each gates psum tile [128g, 512b] accumulates
    4 batch_H K-chunks + 1 one-hot embedding matmul + 4 h K-chunks (t>0).
  - Embedding via one-hot matmul: G' = emb_table @ W_ih[:,IN:].T + b_ih + b_hh
    (host precompute, weight-space only); the one-hot is built on device from
    text via a K=1 broadcast matmul + is_equal against iota -- exact since
    one-hot entries are 0/1 and text values <= 97 are exact in bf16.
  - All matmuls bf16 (full 1 cyc/row PE rate, FWL weight loads). Keeping ANY
    fp32-class matmul in the program corrupts bf16 matmul weights
    (order-dependent -- the FWL-after-FP32-matmul hazard walrus only guards
    in compile order), so even the tiny broadcast matmul is bf16.
  - Gate nonlinearities on ScalarE from PSUM (sigmoid/tanh share one ACT
    table set); cell-state math in fp32 on VectorE; h stored fp32 for the
    output path with a separate bf16 copy feeding the recurrence matmuls.
  - Measured: ~863 us on hardware, PE ~95% busy at ~221 ns per N=512 matmul
    (216 ns streaming floor); outputs rel-err ~4e-3, resid_var ~1e-5.
"""
import sys

for _p in ("/opt/trn_rl_repo",):
    if _p not in sys.path:
        sys.path.insert(0, _p)

import numpy as np
import ml_dtypes

# --- NTFF profile hook shim: the image's antenv lacks axon_hooks; inject a
# module so run_bass_kernel_spmd(trace=True) can profile via the axon .so. ---
import types as _types

def _make_axon_hooks():
    mod = _types.ModuleType("antenv.axon_hooks")
    state = {"hook": None}

    def set_axon_ntff_profile_hook(hook):
        state["hook"] = hook

    def get_axon_ntff_profile_hook():
        if state["hook"] is None:
            try:
                if "/root/.axon_site" not in sys.path:
                    sys.path.insert(0, "/root/.axon_site")
                from trn_agent_boot.trn_boot import _ntff_profile_via_ctypes
                state["hook"] = _ntff_profile_via_ctypes("/opt/axon/libaxon_pjrt.so")
            except Exception:
                return None
        return state["hook"]

    mod.set_axon_ntff_profile_hook = set_axon_ntff_profile_hook
    mod.get_axon_ntff_profile_hook = get_axon_ntff_profile_hook
    return mod

if "antenv.axon_hooks" not in sys.modules:
    sys.modules["antenv.axon_hooks"] = _make_axon_hooks()

import concourse.bacc as bacc
import concourse.mybir as mybir
import concourse.tile as tile
from concourse.bass_utils import run_bass_kernel_spmd

B, T, IN, H, NC_, NE = 4096, 26, 512, 512, 97, 256
NCORES = 8
BL = B // NCORES      # 512 batch rows per core
G4 = 4 * H            # 2048 gate width
NCV = NC_ + 1         # 98 vocab (embedding rows)
KI = IN // 128        # 4 K-chunks over encoder features
KH = H // 128         # 4 K-chunks over hidden
MB = BL // 128        # 4 batch partition tiles
GT = G4 // 128        # 16 gate partition tiles

F32 = mybir.dt.float32
F32R = mybir.dt.float32r
I32 = mybir.dt.int32
AF = mybir.ActivationFunctionType
OP = mybir.AluOpType

USE_BF16 = True
BF16 = mybir.dt.bfloat16
MMD = BF16 if USE_BF16 else F32R

TRACE = False
LAST_RESULT = None
_NC = None


def _build():
    nc = bacc.Bacc("TRN2", target_bir_lowering=False, debug=False,
                   enable_asserts=True, num_devices=NCORES)

    bh_d = nc.dram_tensor("bh", [T, IN, BL], MMD, kind="ExternalInput")
    textT_d = nc.dram_tensor("textT", [1, T, BL], MMD, kind="ExternalInput")
    wx_d = nc.dram_tensor("wx", [KI, 128, G4], MMD, kind="ExternalInput")
    whh_d = nc.dram_tensor("whh", [KH, 128, G4], MMD, kind="ExternalInput")
    gp_d = nc.dram_tensor("gp", [NCV, G4], MMD, kind="ExternalInput")
    wg_d = nc.dram_tensor("wg", [KH, 128, NC_], MMD, kind="ExternalInput")
    bgen_d = nc.dram_tensor("bgen", [NC_, 1], F32, kind="ExternalInput")
    iota_d = nc.dram_tensor("iota", [NCV, 1], F32, kind="ExternalInput")
    ones_d = nc.dram_tensor("ones", [1, NCV], MMD, kind="ExternalInput")
    
    probs_d = nc.dram_tensor("probs", [T, NC_, BL], F32, kind="ExternalOutput")
    hid_d = nc.dram_tensor("hid", [T, H, BL], F32, kind="ExternalOutput")

    with tile.TileContext(nc) as tc:
        with (
            tc.tile_pool(name="wpool", bufs=1) as wp,
            tc.tile_pool(name="sb", bufs=2) as sb,
            tc.tile_pool(name="ps", bufs=1, space="PSUM") as ps,
        ):
            # ---- prologue: step-0 inputs first so PE can start ASAP;
            # whh is not needed until t=1, so its DMA goes last. ----
            iota = wp.tile([NCV, 1], F32)
            nc.sync.dma_start(iota[:], iota_d.ap())
            ones = wp.tile([1, NCV], MMD)
            nc.sync.dma_start(ones[:], ones_d.ap())
            bhT0 = sb.tile([128, KI, BL], MMD, tag="bhT", bufs=2)
            nc.sync.dma_start(
                bhT0[:], bh_d.ap()[0].rearrange("(k p) b -> p k b", k=KI))
            txt0 = sb.tile([NCV, BL], MMD, tag="txt", bufs=2)
            nc.gpsimd.dma_start(
                txt0[:], textT_d.ap()[:, 0, :].partition_broadcast(NCV))
            wx = wp.tile([128, KI, G4], MMD)
            for k in range(KI):
                nc.sync.dma_start(wx[:, k, :], wx_d.ap()[k])
            gp = wp.tile([NCV, G4], MMD)
            nc.sync.dma_start(gp[:], gp_d.ap())
            wg = wp.tile([128, KH, NC_], MMD)
            nc.sync.dma_start(wg[:], wg_d.ap().rearrange("k p c -> p k c"))
            bgen = wp.tile([NC_, 1], F32)
            nc.sync.dma_start(bgen[:], bgen_d.ap())
            whh = wp.tile([128, KH, G4], MMD)
            for k in range(KH):
                nc.sync.dma_start(whh[:, k, :], whh_d.ap()[k])


            h_prev = None
            c_prev = None
            hm_prev = None
            for t in range(T):
                # ---- load batch_H_T[t] directly in matmul layout [i, b] ----
                if t == 0:
                    bhT = bhT0
                    txt = txt0
                else:
                    bhT = sb.tile([128, KI, BL], MMD, tag="bhT", bufs=2)
                    nc.sync.dma_start(
                        bhT[:], bh_d.ap()[t].rearrange("(k p) b -> p k b", k=KI))
                    txt = sb.tile([NCV, BL], MMD, tag="txt", bufs=2)
                    nc.gpsimd.dma_start(
                        txt[:], textT_d.ap()[:, t, :].partition_broadcast(NCV))

                # ---- one-hot of text[:, t] over vocab ----
                oh = sb.tile([NCV, BL], MMD, tag="oh", bufs=2)
                nc.vector.tensor_scalar(oh[:], txt[:], iota[:], None, OP.is_equal)

                # ---- gates: [2048g, 512b] in 16 psum tiles of [128, 512] ----
                # gate order along g: i(0-3) f(4-7) g(8-11) o(12-15)
                sg = {}
                for gt in range(GT):
                    gs = slice(128 * gt, 128 * (gt + 1))
                    acc = ps.tile([128, BL], F32, tag="gates", bufs=7)
                    for k in range(KI):
                        nc.tensor.matmul(acc[:], wx[:, k, gs], bhT[:, k, :],
                                         start=(k == 0), stop=False)
                    nc.tensor.matmul(acc[:], gp[:, gs], oh[:],
                                     start=False, stop=(t == 0))
                    if t > 0:
                        for k in range(KH):
                            nc.tensor.matmul(acc[:], whh[:, k, gs], hm_prev[k][:],
                                             start=False, stop=(k == KH - 1))
                    func = AF.Tanh if gt // 4 == 2 else AF.Sigmoid
                    s = sb.tile([128, BL], F32, tag="sg", bufs=16)
                    nc.scalar.activation(s[:], acc[:], func)
                    sg[gt] = s

                # ---- elementwise LSTM cell update per h-chunk ----
                h_new = []
                c_new = []
                hm_new = []
                for j in range(KH):
                    si, sf, tg, so = sg[j], sg[4 + j], sg[8 + j], sg[12 + j]
                    cn = sb.tile([128, BL], F32, tag="c", bufs=8)
                    if t == 0:
                        nc.vector.tensor_tensor(cn[:], si[:], tg[:], OP.mult)
                    else:
                        t1 = sb.tile([128, BL], F32, tag="t1", bufs=2)
                        nc.vector.tensor_tensor(t1[:], sf[:], c_prev[j][:], OP.mult)
                        t2 = sb.tile([128, BL], F32, tag="t2", bufs=2)
                        nc.vector.tensor_tensor(t2[:], si[:], tg[:], OP.mult)
                        nc.vector.tensor_tensor(cn[:], t1[:], t2[:], OP.add)
                    tc_ = sb.tile([128, BL], F32, tag="tc", bufs=3)
                    nc.scalar.activation(tc_[:], cn[:], AF.Tanh)
                    h = sb.tile([128, BL], F32, tag="h", bufs=8)
                    nc.vector.tensor_tensor(h[:], so[:], tc_[:], OP.mult)
                    hm = sb.tile([128, BL], MMD, tag="hm", bufs=8)
                    nc.vector.tensor_copy(hm[:], h[:])
                    h_new.append(h)
                    c_new.append(cn)
                    hm_new.append(hm)

                # ---- probs[t] = (W_gen @ h + b_gen) in [c, b] layout ----
                pp = ps.tile([NC_, BL], F32, tag="pp", bufs=1)
                for j in range(KH):
                    nc.tensor.matmul(pp[:], wg[:, j, :], hm_new[j][:],
                                     start=(j == 0), stop=(j == KH - 1))
                pb = sb.tile([NC_, BL], F32, tag="pb", bufs=2)
                nc.vector.tensor_scalar(pb[:], pp[:], bgen[:], None, OP.add)
                nc.sync.dma_start(probs_d.ap()[t], pb[:])

                # ---- hs[t] stored transposed [h, b]; host unshard transposes ----
                for j in range(KH):
                    nc.sync.dma_start(
                        hid_d.ap()[t, 128 * j:128 * (j + 1), :], h_new[j][:])

                h_prev, c_prev, hm_prev = h_new, c_new, hm_new

    nc.compile()
    return nc


def _get_nc():
    global _NC
    if _NC is None:
        _NC = _build()
    return _NC


def kernel(batch_H, text, emb_table, W_ih, W_hh, b_ih, b_hh, W_gen, b_gen):
    global LAST_RESULT
    batch_H = np.asarray(batch_H, dtype=np.float32)
    text = np.asarray(text, dtype=np.int32)
    emb_table = np.asarray(emb_table, dtype=np.float32)
    W_ih = np.asarray(W_ih, dtype=np.float32)
    W_hh = np.asarray(W_hh, dtype=np.float32)
    b_ih = np.asarray(b_ih, dtype=np.float32)
    b_hh = np.asarray(b_hh, dtype=np.float32)
    W_gen = np.asarray(W_gen, dtype=np.float32)
    b_gen = np.asarray(b_gen, dtype=np.float32)

    # weight-space host precompute (batch-independent)
    mmnp = ml_dtypes.bfloat16 if USE_BF16 else np.float32
    wx_h = np.ascontiguousarray(W_ih[:, :IN].T.reshape(KI, 128, G4)).astype(mmnp)
    whh_h = np.ascontiguousarray(W_hh.T.reshape(KH, 128, G4)).astype(mmnp)
    gp_h = np.ascontiguousarray(
        emb_table @ W_ih[:, IN:].T + b_ih[None, :] + b_hh[None, :]).astype(mmnp)
    wg_h = np.ascontiguousarray(W_gen.T.reshape(KH, 128, NC_)).astype(mmnp)
    bgen_h = np.ascontiguousarray(b_gen.reshape(NC_, 1))
    iota_h = np.arange(NCV, dtype=np.float32).reshape(NCV, 1)
    ones_h = np.ones((1, NCV), dtype=mmnp)

    in_maps = []
    for c in range(NCORES):
        bsl = slice(c * BL, (c + 1) * BL)
        in_maps.append(dict(
            bh=np.ascontiguousarray(batch_H[bsl].transpose(1, 2, 0)).astype(mmnp),
            textT=np.ascontiguousarray(text[bsl].T.astype(mmnp)).reshape(1, T, BL),
            wx=wx_h, whh=whh_h, gp=gp_h, wg=wg_h, bgen=bgen_h,
            iota=iota_h, ones=ones_h,
        ))

    nc = _get_nc()
    res = run_bass_kernel_spmd(nc, in_maps, list(range(NCORES)), trace=TRACE)
    LAST_RESULT = res

    # device layout is [T, C|H, BL]; unshard transposes back to [B, T, C|H]
    probs = np.concatenate(
        [res.results[c]["probs"].transpose(2, 0, 1) for c in range(NCORES)], axis=0)
    hid = np.concatenate(
        [res.results[c]["hid"].transpose(2, 0, 1) for c in range(NCORES)], axis=0)
    return np.ascontiguousarray(probs, dtype=np.float32), np.ascontiguousarray(hid, dtype=np.float32)


# revision 16
# speedup vs baseline: 1.0124x; 1.0040x over previous
"""Trainium2 Bass kernel for the LSTM decoder problem.

Model (per reference):
    emb = emb_table[text]                       # [B, T, NE]
    x = concat(batch_H, emb)                    # [B, T, IN+NE]
    gx = einsum('bti,gi->tbg', x, W_ih) + b_ih  # [T, B, 4H]
    (h, c) LSTM recurrence over T=26 steps (PyTorch gate order i,f,g,o)
    probs = hs @ W_gen.T + b_gen
    returns (probs [B,T,NC], hs [B,T,H])

Strategy:
  - Data-parallel: batch 4096 -> 512 rows per core across 8 NeuronCores.
  - "Transposed" activation layout on-chip: gates/h/probs as [feature, batch]
    so h feeds the next step's matmul directly -- zero transposes anywhere on
    the device; batch_H shards are shipped [T, IN, BL] and the hid/probs
    outputs come back [T, feat, BL] and are transposed during host unshard.
  - Per-step fused gx: each gates psum tile [128g, 512b] accumulates
    4 batch_H K-chunks + 1 one-hot embedding matmul + 4 h K-chunks (t>0).
  - Embedding via one-hot matmul: G' = emb_table @ W_ih[:,IN:].T + b_ih + b_hh
    (host precompute, weight-space only); the one-hot is built on device from
    text via a K=1 broadcast matmul + is_equal against iota -- exact since
    one-hot entries are 0/1 and text values <= 97 are exact in bf16.
  - All matmuls bf16 (full 1 cyc/row PE rate, FWL weight loads). Keeping ANY
    fp32-class matmul in the program corrupts bf16 matmul weights
    (order-dependent -- the FWL-after-FP32-matmul hazard walrus only guards
    in compile order), so even the tiny broadcast matmul is bf16.
  - Gate nonlinearities on ScalarE from PSUM (sigmoid/tanh share one ACT
    table set); cell-state math in fp32 on VectorE; h stored fp32 for the
    output path with a separate bf16 copy feeding the recurrence matmuls.
  - Measured: ~863 us on hardware, PE ~95% busy at ~221 ns per N=512 matmul
    (216 ns streaming floor); outputs rel-err ~4e-3, resid_var ~1e-5.
"""
import sys

for _p in ("/opt/trn_rl_repo",):
    if _p not in sys.path:
        sys.path.insert(0, _p)

import numpy as np
import ml_dtypes

# --- NTFF profile hook shim: the image's antenv lacks axon_hooks; inject a
# module so run_bass_kernel_spmd(trace=True) can profile via the axon .so. ---
import types as _types

def _make_axon_hooks():
    mod = _types.ModuleType("antenv.axon_hooks")
    state = {"hook": None}

    def set_axon_ntff_profile_hook(hook):
        state["hook"] = hook

    def get_axon_ntff_profile_hook():
        if state["hook"] is None:
            try:
                if "/root/.axon_site" not in sys.path:
                    sys.path.insert(0, "/root/.axon_site")
                from trn_agent_boot.trn_boot import _ntff_profile_via_ctypes
                state["hook"] = _ntff_profile_via_ctypes("/opt/axon/libaxon_pjrt.so")
            except Exception:
                return None
        return state["hook"]

    mod.set_axon_ntff_profile_hook = set_axon_ntff_profile_hook
    mod.get_axon_ntff_profile_hook = get_axon_ntff_profile_hook
    return mod

if "antenv.axon_hooks" not in sys.modules:
    sys.modules["antenv.axon_hooks"] = _make_axon_hooks()

import concourse.bacc as bacc
import concourse.mybir as mybir
import concourse.tile as tile
from concourse.bass_utils import run_bass_kernel_spmd

B, T, IN, H, NC_, NE = 4096, 26, 512, 512, 97, 256
NCORES = 8
BL = B // NCORES      # 512 batch rows per core
G4 = 4 * H            # 2048 gate width
NCV = NC_ + 1         # 98 vocab (embedding rows)
KI = IN // 128        # 4 K-chunks over encoder features
KH = H // 128         # 4 K-chunks over hidden
MB = BL // 128        # 4 batch partition tiles
GT = G4 // 128        # 16 gate partition tiles

F32 = mybir.dt.float32
F32R = mybir.dt.float32r
I32 = mybir.dt.int32
AF = mybir.ActivationFunctionType
OP = mybir.AluOpType

USE_BF16 = True
BF16 = mybir.dt.bfloat16
MMD = BF16 if USE_BF16 else F32R

TRACE = False
LAST_RESULT = None
_NC = None


def _build():
    nc = bacc.Bacc("TRN2", target_bir_lowering=False, debug=False,
                   enable_asserts=True, num_devices=NCORES)

    bh_d = nc.dram_tensor("bh", [T, IN, BL], MMD, kind="ExternalInput")
    textT_d = nc.dram_tensor("textT", [1, T, BL], MMD, kind="ExternalInput")
    wx_d = nc.dram_tensor("wx", [KI, 128, G4], MMD, kind="ExternalInput")
    whh_d = nc.dram_tensor("whh", [KH, 128, G4], MMD, kind="ExternalInput")
    gp_d = nc.dram_tensor("gp", [NCV, G4], MMD, kind="ExternalInput")
    wg_d = nc.dram_tensor("wg", [KH, 128, NC_], MMD, kind="ExternalInput")
    bgen_d = nc.dram_tensor("bgen", [NC_, 1], F32, kind="ExternalInput")
    iota_d = nc.dram_tensor("iota", [NCV, 1], F32, kind="ExternalInput")
    
    probs_d = nc.dram_tensor("probs", [T, NC_, BL], F32, kind="ExternalOutput")
    hid_d = nc.dram_tensor("hid", [T, H, BL], F32, kind="ExternalOutput")

    with tile.TileContext(nc) as tc:
        with (
            tc.tile_pool(name="wpool", bufs=1) as wp,
            tc.tile_pool(name="sb", bufs=2) as sb,
            tc.tile_pool(name="ps", bufs=1, space="PSUM") as ps,
        ):
            # ---- prologue: step-0 inputs first so PE can start ASAP;
            # whh is not needed until t=1, so its DMA goes last. ----
            iota = wp.tile([NCV, 1], F32)
            nc.sync.dma_start(iota[:], iota_d.ap())
            bhT0 = sb.tile([128, KI, BL], MMD, tag="bhT", bufs=2)
            nc.sync.dma_start(
                bhT0[:], bh_d.ap()[0].rearrange("(k p) b -> p k b", k=KI))
            txt0 = sb.tile([NCV, BL], MMD, tag="txt", bufs=2)
            nc.gpsimd.dma_start(
                txt0[:], textT_d.ap()[:, 0, :].partition_broadcast(NCV))
            wx = wp.tile([128, KI, G4], MMD)
            for k in range(KI):
                nc.sync.dma_start(wx[:, k, :], wx_d.ap()[k])
            gp = wp.tile([NCV, G4], MMD)
            nc.sync.dma_start(gp[:], gp_d.ap())
            wg = wp.tile([128, KH, NC_], MMD)
            nc.sync.dma_start(wg[:], wg_d.ap().rearrange("k p c -> p k c"))
            bgen = wp.tile([NC_, 1], F32)
            nc.sync.dma_start(bgen[:], bgen_d.ap())
            whh = wp.tile([128, KH, G4], MMD)
            for k in range(KH):
                nc.sync.dma_start(whh[:, k, :], whh_d.ap()[k])


            h_prev = None
            c_prev = None
            hm_prev = None
            for t in range(T):
                # ---- load batch_H_T[t] directly in matmul layout [i, b] ----
                if t == 0:
                    bhT = bhT0
                    txt = txt0
                else:
                    bhT = sb.tile([128, KI, BL], MMD, tag="bhT", bufs=2)
                    nc.sync.dma_start(
                        bhT[:], bh_d.ap()[t].rearrange("(k p) b -> p k b", k=KI))
                    txt = sb.tile([NCV, BL], MMD, tag="txt", bufs=2)
                    nc.gpsimd.dma_start(
                        txt[:], textT_d.ap()[:, t, :].partition_broadcast(NCV))

                # ---- one-hot of text[:, t] over vocab ----
                oh = sb.tile([NCV, BL], MMD, tag="oh", bufs=2)
                nc.vector.tensor_scalar(oh[:], txt[:], iota[:], None, OP.is_equal)

                # ---- gates: [2048g, 512b] in 16 psum tiles of [128, 512] ----
                # gate order along g: i(0-3) f(4-7) g(8-11) o(12-15)
                sg = {}
                for gt in range(GT):
                    gs = slice(128 * gt, 128 * (gt + 1))
                    acc = ps.tile([128, BL], F32, tag="gates", bufs=7)
                    for k in range(KI):
                        nc.tensor.matmul(acc[:], wx[:, k, gs], bhT[:, k, :],
                                         start=(k == 0), stop=False)
                    nc.tensor.matmul(acc[:], gp[:, gs], oh[:],
                                     start=False, stop=(t == 0))
                    if t > 0:
                        for k in range(KH):
                            nc.tensor.matmul(acc[:], whh[:, k, gs], hm_prev[k][:],
                                             start=False, stop=(k == KH - 1))
                    func = AF.Tanh if gt // 4 == 2 else AF.Sigmoid
                    s = sb.tile([128, BL], F32, tag="sg", bufs=16)
                    nc.scalar.activation(s[:], acc[:], func)
                    sg[gt] = s

                # ---- elementwise LSTM cell update per h-chunk ----
                h_new = []
                c_new = []
                hm_new = []
                for j in range(KH):
                    si, sf, tg, so = sg[j], sg[4 + j], sg[8 + j], sg[12 + j]
                    cn = sb.tile([128, BL], F32, tag="c", bufs=8)
                    if t == 0:
                        nc.vector.tensor_tensor(cn[:], si[:], tg[:], OP.mult)
                    else:
                        t1 = sb.tile([128, BL], F32, tag="t1", bufs=2)
                        nc.vector.tensor_tensor(t1[:], sf[:], c_prev[j][:], OP.mult)
                        t2 = sb.tile([128, BL], F32, tag="t2", bufs=2)
                        nc.vector.tensor_tensor(t2[:], si[:], tg[:], OP.mult)
                        nc.vector.tensor_tensor(cn[:], t1[:], t2[:], OP.add)
                    tc_ = sb.tile([128, BL], F32, tag="tc", bufs=3)
                    nc.scalar.activation(tc_[:], cn[:], AF.Tanh)
                    h = sb.tile([128, BL], F32, tag="h", bufs=8)
                    nc.vector.tensor_tensor(h[:], so[:], tc_[:], OP.mult)
                    hm = sb.tile([128, BL], MMD, tag="hm", bufs=8)
                    nc.vector.tensor_copy(hm[:], h[:])
                    h_new.append(h)
                    c_new.append(cn)
                    hm_new.append(hm)

                # ---- probs[t] = (W_gen @ h + b_gen) in [c, b] layout ----
                pp = ps.tile([NC_, BL], F32, tag="pp", bufs=1)
                for j in range(KH):
                    nc.tensor.matmul(pp[:], wg[:, j, :], hm_new[j][:],
                                     start=(j == 0), stop=(j == KH - 1))
                pb = sb.tile([NC_, BL], F32, tag="pb", bufs=2)
                nc.vector.tensor_scalar(pb[:], pp[:], bgen[:], None, OP.add)
                nc.sync.dma_start(probs_d.ap()[t], pb[:])

                # ---- hs[t] stored transposed [h, b]; host unshard transposes ----
                for j in range(KH):
                    nc.sync.dma_start(
                        hid_d.ap()[t, 128 * j:128 * (j + 1), :], h_new[j][:])

                h_prev, c_prev, hm_prev = h_new, c_new, hm_new

    nc.compile()
    return nc


def _get_nc():
    global _NC
    if _NC is None:
        _NC = _build()
    return _NC


def kernel(batch_H, text, emb_table, W_ih, W_hh, b_ih, b_hh, W_gen, b_gen):
    global LAST_RESULT
    batch_H = np.asarray(batch_H, dtype=np.float32)
    text = np.asarray(text, dtype=np.int32)
    emb_table = np.asarray(emb_table, dtype=np.float32)
    W_ih = np.asarray(W_ih, dtype=np.float32)
    W_hh = np.asarray(W_hh, dtype=np.float32)
    b_ih = np.asarray(b_ih, dtype=np.float32)
    b_hh = np.asarray(b_hh, dtype=np.float32)
    W_gen = np.asarray(W_gen, dtype=np.float32)
    b_gen = np.asarray(b_gen, dtype=np.float32)

    # weight-space host precompute (batch-independent)
    mmnp = ml_dtypes.bfloat16 if USE_BF16 else np.float32
    wx_h = np.ascontiguousarray(W_ih[:, :IN].T.reshape(KI, 128, G4)).astype(mmnp)
    whh_h = np.ascontiguousarray(W_hh.T.reshape(KH, 128, G4)).astype(mmnp)
    gp_h = np.ascontiguousarray(
        emb_table @ W_ih[:, IN:].T + b_ih[None, :] + b_hh[None, :]).astype(mmnp)
    wg_h = np.ascontiguousarray(W_gen.T.reshape(KH, 128, NC_)).astype(mmnp)
    bgen_h = np.ascontiguousarray(b_gen.reshape(NC_, 1))
    iota_h = np.arange(NCV, dtype=np.float32).reshape(NCV, 1)

    in_maps = []
    for c in range(NCORES):
        bsl = slice(c * BL, (c + 1) * BL)
        in_maps.append(dict(
            bh=np.ascontiguousarray(batch_H[bsl].transpose(1, 2, 0)).astype(mmnp),
            textT=np.ascontiguousarray(text[bsl].T.astype(mmnp)).reshape(1, T, BL),
            wx=wx_h, whh=whh_h, gp=gp_h, wg=wg_h, bgen=bgen_h,
            iota=iota_h,
        ))

    nc = _get_nc()
    res = run_bass_kernel_spmd(nc, in_maps, list(range(NCORES)), trace=TRACE)
    LAST_RESULT = res

    # device layout is [T, C|H, BL]; unshard transposes back to [B, T, C|H]
    probs = np.concatenate(
        [res.results[c]["probs"].transpose(2, 0, 1) for c in range(NCORES)], axis=0)
    hid = np.concatenate(
        [res.results[c]["hid"].transpose(2, 0, 1) for c in range(NCORES)], axis=0)
    return np.ascontiguousarray(probs, dtype=np.float32), np.ascontiguousarray(hid, dtype=np.float32)
